# revision 8
# baseline (speedup 1.0000x reference)
"""LRU (Linear Recurrent Unit) block kernel for Trainium2, 8 NeuronCores.

Math (per batch element b, see reference):
    lam  = exp(-exp(nu_log)) * exp(i*exp(theta_log))          (S,) complex
    Bn   = (B_re + i B_im) * exp(gamma_log)[:, None]          (S, D)
    Bu_t = Bn @ x_t                                           complex
    s_t  = lam * s_{t-1} + Bu_t                               diagonal complex scan
    z_t  = Re(C s_t) + D x_t
    out  = W_proj @ gelu(W_fc @ z + b_fc) + b_proj + x        (MLP + residual)

Device strategy (data-parallel over batch, 2 sequences/core):
  - Everything runs transposed: features on SBUF partitions, tokens on the
    free axis. x is pre-transposed on the host (fp32 DMA-transpose is
    unsupported and PE transposes would waste cycles).
  - The complex scan uses the modulus-phase decomposition: with
    lam = r*e^{i*theta}, v_tau = e^{-i*theta*tau} * s_tau satisfies
    v_tau = r * v_{tau-1} + e^{-i*theta*tau} * Bu_tau, i.e. TWO REAL
    first-order recurrences, which map onto the DVE tensor_tensor_scan
    instruction. cos/sin twiddle tables over a chunk of 256 positions are
    precomputed on the host in float64. Chunk carries are rotated by
    e^{i*theta*256} (per-state constants) between chunks.
  - All matmuls run as float32r (full-speed PE path, 1 cycle/row).
"""

import numpy as np

import concourse.bass as bass
import concourse.mybir as mybir
import concourse.tile as tile
from concourse.vector_clock import ScopedClock
from concourse.bass_utils import run_bass_kernel_spmd

Alu = mybir.AluOpType
F32 = mybir.dt.float32
F32R = mybir.dt.float32r
ACTF = mybir.ActivationFunctionType
GELU_FUNC = ACTF.Gelu  # overridable for CoreSim (no Gelu in the interpreter)

BATCH, SEQLEN, DM, DS, DF = 16, 8192, 256, 256, 1024
NCORES = 8
NSEQ = BATCH // NCORES          # sequences per core
PC = 256                        # positions per chunk (per sequence)

# ---- consts blob layout (columns of a [128, NCOL] fp32 tensor) -------------
NW = 52                         # 128-col weight tiles
COS0 = NW * 128                 # cos table  [st][tau]  2*PC cols
SIN0 = COS0 + 2 * PC
RT0 = SIN0 + 2 * PC             # r (decay) broadcast over tau, 2*PC cols
ROT0 = RT0 + 2 * PC             # carry rotation: [rotc0, rotc1, rots0, rots1]
BFC0 = ROT0 + 4                 # fc1 bias per f-tile, 8 cols
BPJ0 = BFC0 + 8                 # proj bias per o-tile, 2 cols
NCOL = BPJ0 + 2

# weight tile index helpers
def _wi_bnre(kt, st): return 0 + kt * 2 + st
def _wi_bnim(kt, st): return 4 + kt * 2 + st
def _wi_cr(st, ot):   return 8 + st * 2 + ot
def _wi_cm(st, ot):   return 12 + st * 2 + ot
def _wi_dT(kt, ot):   return 16 + kt * 2 + ot
def _wi_wfc(kt, ft):  return 20 + kt * 8 + ft
def _wi_wpj(ft, ot):  return 36 + ft * 2 + ot


# --- tile-exit drain workaround: walrus in this container caps the sync-wait
# slots on a TPB_CTRL Drain; split the waits onto follow-up SP nops. ---------
def _patched_drain_and_barrier(self, tick_clock, wait_clock):
    nc = self.nc
    drain_inst = nc.sync.drain()
    wait_clock.add_sem_waits(
        drain_inst.ins, ScopedClock({None: tick_clock.global_clock})
    )
    si = drain_inst.ins.sync_info
    if si is not None and si.on_wait and len(si.on_wait) > 1:
        waits = list(si.on_wait)
        drain_inst.ins.sync_info = mybir.SyncInfo(
            on_wait=[waits[0]], on_update=list(si.on_update or [])
        )
        for w in waits[1:]:
            nop = nc.sync.nop(hint="drain_wait_split", nofuse=True)
            nop.ins.sync_info = mybir.SyncInfo(on_wait=[w], on_update=[])
    nc.all_engine_barrier()
    assert self.sems is not None
    popped = nc._tile_sem_poison_stack.pop()
    assert popped is self._sem_poison
    nc.clear_and_free_semaphores(list(self.sems.allocated().values()))
    nc.all_engine_barrier()


tile.TileContext._drain_and_barrier = _patched_drain_and_barrier


# --- universal sync-wait splitter: this container's walrus rejects >1 wait on
# several instruction structs (S3_LW matmul, TPB_CTRL drain, ...). Rewrite the
# serialized BIR so every instruction carries at most one wait; extra waits
# move to injected same-engine NoOps placed immediately before it. -----------
def _split_sync_waits(bir: bytes) -> bytes:
    import json as _json

    m = _json.loads(bir)
    ctr = 0
    for f in m.get("functions", []):
        for bb in f.get("blocks", []):
            insts = bb.get("instructions")
            if not insts:
                continue
            out = []
            for inst in insts:
                si = inst.get("sync_info")
                ow = (si or {}).get("on_wait") or []
                if len(ow) > 1:
                    for wdesc in ow[:-1]:
                        ctr += 1
                        out.append({
                            "engine": inst["engine"],
                            "ins": [],
                            "outs": [],
                            "name": f"I-wsplit{ctr}",
                            "opcode": "NoOp",
                            "sync_info": {"on_update": [], "on_wait": [wdesc]},
                            "text_hint": "wait_split",
                        })
                    si["on_wait"] = [ow[-1]]
                out.append(inst)
            bb["instructions"] = out
    return _json.dumps(m).encode()


_orig_to_json_bytes = bass.Bass.to_json_bytes


def _to_json_bytes_split(self):
    return _split_sync_waits(_orig_to_json_bytes(self))


bass.Bass.to_json_bytes = _to_json_bytes_split


def _enable_axon_ntff_profiling():
    """Best-effort: register the axon NTFF profile hook (the image's antenv
    lacks axon_hooks; the backing ctypes impl ships in trn_agent_boot) and
    neuter the S3 artifact upload that the trace path would attempt."""
    try:
        import sys, types
        try:
            import antenv.axon_hooks  # noqa: F401
        except ImportError:
            mod = types.ModuleType("antenv.axon_hooks")
            mod._hook = None

            def set_axon_ntff_profile_hook(h):
                mod._hook = h

            def get_axon_ntff_profile_hook():
                return mod._hook

            mod.set_axon_ntff_profile_hook = set_axon_ntff_profile_hook
            mod.get_axon_ntff_profile_hook = get_axon_ntff_profile_hook
            sys.modules["antenv.axon_hooks"] = mod
            import antenv
            antenv.axon_hooks = mod
        import antenv.axon_hooks as ah
        if ah.get_axon_ntff_profile_hook() is None:
            from trn_agent_boot.trn_boot import _ntff_profile_via_ctypes
            ah.set_axon_ntff_profile_hook(
                _ntff_profile_via_ctypes("/opt/axon/libaxon_pjrt.so")
            )
        import concourse.bass_utils as bu
        bu.upload_artifacts = lambda tmpdir: ""
    except Exception:
        pass


import os as _os
if _os.environ.get("BASS_TRACE"):
    _enable_axon_ntff_profiling()


def build_nc(seqlen=SEQLEN, nseq=NSEQ, pc=PC):
    """Build the per-core Bass module. Token layout: [nseq, seqlen] flattened."""
    ntok = nseq * seqlen
    nchunk = seqlen // pc
    assert seqlen % pc == 0

    nc = bass.Bass()
    xT = nc.declare_dram_parameter("xT", [2, 128, ntok], F32R, isOutput=False)
    consts = nc.declare_dram_parameter("consts", [128, NCOL], F32R, isOutput=False)
    outT = nc.declare_dram_parameter("outT", [2, 128, ntok], F32, isOutput=True)

    xTv = [xT[kt].rearrange("p (b l) -> p b l", b=nseq) for kt in range(2)]
    outTv = [outT[ot].rearrange("p (b l) -> p b l", b=nseq) for ot in range(2)]

    from contextlib import ExitStack
    with tile.TileContext(nc) as tc, ExitStack() as ctx:
        singles = ctx.enter_context(tc.tile_pool(name="singles", bufs=1))
        work = ctx.enter_context(tc.tile_pool(name="work", bufs=2))
        tmps = ctx.enter_context(tc.tile_pool(name="tmps", bufs=4))
        carries = ctx.enter_context(tc.tile_pool(name="carries", bufs=2))
        ps_bu = ctx.enter_context(tc.tile_pool(name="ps_bu", bufs=2, space="PSUM"))
        ps_z = ctx.enter_context(tc.tile_pool(name="ps_z", bufs=2, space="PSUM"))
        ps_h = ctx.enter_context(tc.tile_pool(name="ps_h", bufs=2, space="PSUM"))
        ps_q = ctx.enter_context(tc.tile_pool(name="ps_q", bufs=2, space="PSUM"))

        cb = singles.tile([128, NCOL], F32R, tag="consts")
        nc.sync.dma_start(out=cb[:], in_=consts[:])

        def w(i):  # weight tile i as fp32r lhsT [128, 128]
            return cb[:, i * 128:(i + 1) * 128]

        def tab3(base, st):  # [128, nseq, pc] broadcast view of a table row-block
            return cb[:, base + st * pc: base + (st + 1) * pc].bitcast(F32)[:, None, :].to_broadcast(
                [128, nseq, pc]
            )

        def tab2(base, st):  # [128, pc] 2D view (scan data0)
            return cb[:, base + st * pc: base + (st + 1) * pc].bitcast(F32)

        # carry state: [128, st, plane, b]; zero-init
        carry = carries.tile([128, 2, 2, nseq], F32, tag="carry")
        nc.vector.memset(carry[:], 0.0)

        for c in range(nchunk):
            lo, hi = c * pc, (c + 1) * pc

            # ---- load x^T chunk (both k-tiles) -----------------------------
            xt = []
            for kt in range(2):
                t = work.tile([128, nseq, pc], F32R, tag=f"xt{kt}")
                nc.sync.dma_start(out=t[:], in_=xTv[kt][:, :, lo:hi])
                xt.append(t)

            # ---- Bu = Bn @ x^T (complex), then twiddle + scans + untwiddle -
            s_re, s_im, v_re, v_im = [], [], [], []
            for st in range(2):
                bu_re = ps_bu.tile([128, nseq, pc], F32, tag="bu")
                bu_im = ps_bu.tile([128, nseq, pc], F32, tag="bu")
                for plane, ps in ((0, bu_re), (1, bu_im)):
                    for kt in range(2):
                        wi = _wi_bnre(kt, st) if plane == 0 else _wi_bnim(kt, st)
                        nc.tensor.matmul(
                            ps[:], w(wi), xt[kt][:],
                            start=(kt == 0), stop=(kt == 1),
                        )

                cosb, sinb = tab3(COS0, st), tab3(SIN0, st)
                # ut = e^{-i theta tau} * Bu
                ut_re = work.tile([128, nseq, pc], F32, tag=f"utre{st}")
                ut_im = work.tile([128, nseq, pc], F32, tag=f"utim{st}")
                t1 = tmps.tile([128, nseq, pc], F32, tag="twtmp")
                t2 = tmps.tile([128, nseq, pc], F32, tag="twtmp")
                nc.vector.tensor_tensor(t1[:], cosb, bu_re[:], Alu.mult)
                nc.vector.tensor_tensor(t2[:], sinb, bu_im[:], Alu.mult)
                nc.vector.tensor_tensor(ut_re[:], t1[:], t2[:], Alu.add)
                t3 = tmps.tile([128, nseq, pc], F32, tag="twtmp")
                t4 = tmps.tile([128, nseq, pc], F32, tag="twtmp")
                nc.vector.tensor_tensor(t3[:], cosb, bu_im[:], Alu.mult)
                nc.vector.tensor_tensor(t4[:], sinb, bu_re[:], Alu.mult)
                nc.vector.tensor_tensor(ut_im[:], t3[:], t4[:], Alu.subtract)

                # real scans: v = r*v_prev + ut   (per seq, per plane)
                vr = work.tile([128, nseq, pc], F32, tag=f"vre{st}")
                vi = work.tile([128, nseq, pc], F32, tag=f"vim{st}")
                rt = tab2(RT0, st)
                for b in range(nseq):
                    nc.vector.tensor_tensor_scan(
                        vr[:, b, :], rt, ut_re[:, b, :],
                        carry[:, st, 0, b:b + 1], Alu.mult, Alu.add,
                    )
                    nc.vector.tensor_tensor_scan(
                        vi[:, b, :], rt, ut_im[:, b, :],
                        carry[:, st, 1, b:b + 1], Alu.mult, Alu.add,
                    )
                v_re.append(vr)
                v_im.append(vi)

                # untwiddle: s = e^{+i theta tau} * v
                sr = work.tile([128, nseq, pc], F32R, tag=f"sre{st}")
                si_ = work.tile([128, nseq, pc], F32R, tag=f"sim{st}")
                u1 = tmps.tile([128, nseq, pc], F32, tag="twtmp")
                u2 = tmps.tile([128, nseq, pc], F32, tag="twtmp")
                nc.vector.tensor_tensor(u1[:], cosb, vr[:], Alu.mult)
                nc.vector.tensor_tensor(u2[:], sinb, vi[:], Alu.mult)
                nc.vector.tensor_tensor(sr[:], u1[:], u2[:], Alu.subtract)
                u3 = tmps.tile([128, nseq, pc], F32, tag="twtmp")
                u4 = tmps.tile([128, nseq, pc], F32, tag="twtmp")
                nc.vector.tensor_tensor(u3[:], cosb, vi[:], Alu.mult)
                nc.vector.tensor_tensor(u4[:], sinb, vr[:], Alu.mult)
                nc.vector.tensor_tensor(si_[:], u3[:], u4[:], Alu.add)
                s_re.append(sr)
                s_im.append(si_)

            # ---- carry update: c_next = e^{i theta pc} * v[:, :, -1] -------
            carry_new = carries.tile([128, 2, 2, nseq], F32, tag="carry")
            for st in range(2):
                rotc = cb[:, ROT0 + st: ROT0 + st + 1].bitcast(F32)
                rots = cb[:, ROT0 + 2 + st: ROT0 + 3 + st].bitcast(F32)
                vrl = v_re[st][:, :, pc - 1]
                vil = v_im[st][:, :, pc - 1]
                ta = tmps.tile([128, nseq], F32, tag="cartmp")
                nc.vector.tensor_scalar_mul(ta[:], vil, rots)
                nc.vector.scalar_tensor_tensor(
                    carry_new[:, st, 0, :], vrl, rotc, ta[:], Alu.mult, Alu.subtract
                )
                tb = tmps.tile([128, nseq], F32, tag="cartmp")
                nc.vector.tensor_scalar_mul(tb[:], vrl, rots)
                nc.vector.scalar_tensor_tensor(
                    carry_new[:, st, 1, :], vil, rotc, tb[:], Alu.mult, Alu.add
                )
            carry = carry_new

            # ---- z = C_re s_re - C_im s_im + D x ---------------------------
            z_sb = []
            for ot in range(2):
                zp = ps_z.tile([128, nseq, pc], F32, tag="z")
                for st in range(2):
                    nc.tensor.matmul(zp[:], w(_wi_cr(st, ot)),
                                     s_re[st][:],
                                     start=(st == 0), stop=False)
                for st in range(2):
                    nc.tensor.matmul(zp[:], w(_wi_cm(st, ot)),
                                     s_im[st][:],
                                     start=False, stop=False)
                for kt in range(2):
                    nc.tensor.matmul(zp[:], w(_wi_dT(kt, ot)),
                                     xt[kt][:],
                                     start=False, stop=(kt == 1))
                zs = work.tile([128, nseq, pc], F32R, tag=f"z{ot}")
                nc.scalar.activation(zs[:], zp[:], ACTF.Copy)
                z_sb.append(zs)

            # ---- h = gelu(W_fc^T z + b_fc) ---------------------------------
            h_sb = []
            for ft in range(8):
                hp = ps_h.tile([128, nseq, pc], F32, tag="h")
                for kt in range(2):
                    nc.tensor.matmul(hp[:], w(_wi_wfc(kt, ft)),
                                     z_sb[kt][:],
                                     start=(kt == 0), stop=(kt == 1))
                hs = work.tile([128, nseq, pc], F32R, tag=f"h{ft}")
                nc.scalar.activation(
                    hs[:], hp[:], GELU_FUNC,
                    bias=cb[:, BFC0 + ft: BFC0 + ft + 1].bitcast(F32), scale=1.0,
                )
                h_sb.append(hs)

            # ---- out = W_proj^T h + b_proj + x (residual) ------------------
            for ot in range(2):
                qp = ps_q.tile([128, nseq, pc], F32, tag="q")
                for ft in range(8):
                    nc.tensor.matmul(qp[:], w(_wi_wpj(ft, ot)),
                                     h_sb[ft][:],
                                     start=(ft == 0), stop=(ft == 7))
                ob = work.tile([128, nseq, pc], F32, tag=f"ob{ot}")
                nc.vector.scalar_tensor_tensor(
                    ob[:], qp[:], cb[:, BPJ0 + ot: BPJ0 + ot + 1].bitcast(F32), xt[ot][:].bitcast(F32),
                    Alu.add, Alu.add,
                )
                nc.sync.dma_start(out=outTv[ot][:, :, lo:hi], in_=ob[:])
    return nc


def pack_consts(nu_log, theta_log, gamma_log, B_re, B_im, C_re, C_im, D,
                W_fc, b_fc, W_proj, b_proj, pc=PC):
    """Assemble the [128, NCOL] fp32 consts blob (tables in float64)."""
    f8 = np.float64
    nu = np.exp(np.asarray(nu_log, f8))
    r = np.exp(-nu)                      # modulus of lam, (S,)
    theta = np.exp(np.asarray(theta_log, f8))
    gamma = np.exp(np.asarray(gamma_log, f8))
    Bn_re = np.asarray(B_re, f8) * gamma[:, None]
    Bn_im = np.asarray(B_im, f8) * gamma[:, None]
    C_re = np.asarray(C_re, f8)
    C_im = np.asarray(C_im, f8)
    D = np.asarray(D, f8)
    W_fc = np.asarray(W_fc, f8)
    W_proj = np.asarray(W_proj, f8)

    cb = np.zeros((128, NCOL), np.float32)

    def put(i, m):  # weight tile i <- m [128, 128]
        cb[:, i * 128:(i + 1) * 128] = np.asarray(m, np.float32)

    for kt in range(2):
        for st in range(2):
            put(_wi_bnre(kt, st),
                Bn_re[st * 128:(st + 1) * 128, kt * 128:(kt + 1) * 128].T)
            put(_wi_bnim(kt, st),
                Bn_im[st * 128:(st + 1) * 128, kt * 128:(kt + 1) * 128].T)
    for st in range(2):
        for ot in range(2):
            put(_wi_cr(st, ot),
                C_re[ot * 128:(ot + 1) * 128, st * 128:(st + 1) * 128].T)
            put(_wi_cm(st, ot),
                -C_im[ot * 128:(ot + 1) * 128, st * 128:(st + 1) * 128].T)
    for kt in range(2):
        for ot in range(2):
            put(_wi_dT(kt, ot),
                D[ot * 128:(ot + 1) * 128, kt * 128:(kt + 1) * 128].T)
    for kt in range(2):
        for ft in range(8):
            put(_wi_wfc(kt, ft),
                W_fc[kt * 128:(kt + 1) * 128, ft * 128:(ft + 1) * 128])
    for ft in range(8):
        for ot in range(2):
            put(_wi_wpj(ft, ot),
                W_proj[ft * 128:(ft + 1) * 128, ot * 128:(ot + 1) * 128])

    tau = np.arange(pc, dtype=f8)
    for st in range(2):
        th = theta[st * 128:(st + 1) * 128]
        ang = th[:, None] * tau[None, :]
        cb[:, COS0 + st * pc: COS0 + (st + 1) * pc] = np.cos(ang)
        cb[:, SIN0 + st * pc: SIN0 + (st + 1) * pc] = np.sin(ang)
        cb[:, RT0 + st * pc: RT0 + (st + 1) * pc] = r[st * 128:(st + 1) * 128, None]
        cb[:, ROT0 + st] = np.cos(th * pc)
        cb[:, ROT0 + 2 + st] = np.sin(th * pc)
    for ft in range(8):
        cb[:, BFC0 + ft] = np.asarray(b_fc, np.float32)[ft * 128:(ft + 1) * 128]
    for ot in range(2):
        cb[:, BPJ0 + ot] = np.asarray(b_proj, np.float32)[ot * 128:(ot + 1) * 128]
    return cb


_NC_CACHE = {}
LAST_RUN_INFO = {}


def kernel(x, nu_log, theta_log, gamma_log, B_re, B_im, C_re, C_im, D,
           W_fc, b_fc, W_proj, b_proj):
    x = np.asarray(x, np.float32)
    assert x.shape == (BATCH, SEQLEN, DM)

    key = (SEQLEN, NSEQ, PC)
    if key not in _NC_CACHE:
        _NC_CACHE[key] = build_nc(SEQLEN, NSEQ, PC)
    nc = _NC_CACHE[key]

    cb = pack_consts(nu_log, theta_log, gamma_log, B_re, B_im, C_re, C_im, D,
                     W_fc, b_fc, W_proj, b_proj, PC)

    in_maps = []
    for c in range(NCORES):
        xc = x[c * NSEQ:(c + 1) * NSEQ]                      # (nseq, L, D)
        xT = np.ascontiguousarray(
            xc.transpose(2, 0, 1).reshape(2, 128, NSEQ * SEQLEN)
        )
        in_maps.append({"xT": xT, "consts": cb})

    res = run_bass_kernel_spmd(nc, in_maps, core_ids=list(range(NCORES)))
    LAST_RUN_INFO.clear()
    LAST_RUN_INFO.update(
        exec_time_ns=res.exec_time_ns,
        mean_exec_time_ns=res.mean_exec_time_ns,
        trace=res.instructions_and_trace[1] if res.instructions_and_trace else None,
    )

    out = np.empty((BATCH, SEQLEN, DM), np.float32)
    for c in range(NCORES):
        oT = res.results[c]["outT"]                          # (2, 128, ntok)
        out[c * NSEQ:(c + 1) * NSEQ] = (
            oT.reshape(DM, NSEQ, SEQLEN).transpose(1, 2, 0)
        )
    return out
